# revision 39
# baseline (speedup 1.0000x reference)
"""Trainium2 Bass kernel: 5x5 window median+variance denoise filter.

y = relu(x - noise_var/(var5x5(x)+1e-10) * (x - median5x5(x) + noise_bias))
with zero-padded 5x5 windows, unbiased variance (ddof=1).

Sharding: pure data parallel, B=16 images split 2-per-core across 8 cores.

Design (v1 was an fp32 DVE-only comparator network at 668us/core):
  * the median comparator network runs in fp16 on the DVE; the input is
    pre-rounded to fp16 on the host, so tiles are fp16 end-to-end and every
    tensor_tensor min/max gets the DVE 2-byte fast path. Measured on HW:
    [128, 2, 516] fp16 ops run ~1.7x faster than fp32 (the [128, 4, 516]
    shape falls back to 1x - keep ROWS=2). Accuracy verified offline:
    full-fp16 pipeline rel err 1.3e-3 vs the 2e-2 gate (the error
    amplification factor nv/(var+eps) maxes at ~10 on this input).
  * variance path off the DVE almost entirely: vertical 5-row sums via PE
    matmuls against banded weight matrices (host-built, nv folded into the
    weights; fp16 weights for the x-plane, fp32 for the x^2-plane; exact
    fp32 PSUM accumulation); horizontal sliding 5-sums via single DVE
    tensor_tensor_scan ops (state = (Sv[c] + state) - Sv[c-5]); squares,
    PSUM->SBUF copies and fp16 downconverts on the Activation engine.
    All rescaling constants (c2, sqrt(lam), eps') are folded host-side so
    the math matches the reference exactly up to rounding.
  * the two chunks (halves) of an image are emitted as op-interleaved
    DVE streams, and each stage's comparator list is topologically
    re-sorted so consecutive CEs touch disjoint positions (_space_ces).
    Together these give producer->consumer distances >= 4, which lets the
    DVE pipeline ops back-to-back: measured 287us (serial) -> 248us
    (pairwise interleave) -> 144us (interleave + CE spacing). Each parity
    has its own input tiles and variance buffers so the pair never
    serializes.
  * reciprocal via the single-op DVE reciprocal_approx_fast; formula tail
    in fp16 on the DVE. (Measured dead ends: the Pool/GPSIMD engine only
    accepts TensorTensor add/sub/mult + TensorScalar opcodes -- no
    tensor-tensor min/max, so the median cannot be split onto it -- and
    its real elementwise throughput is ~10x below the cost model, so even
    the formula tail is faster on the DVE. ROWS=4 tiles drop the DVE
    fp16 fast path to 1x; keep ROWS=2.)
  * fp16 output store, upcast to fp32 on the host.
"""
import numpy as np

import concourse.bass as bass  # noqa: F401
import concourse.mybir as mybir
from concourse import bacc, tile
from concourse.bass_utils import run_bass_kernel_spmd

F32 = mybir.dt.float32
F16 = mybir.dt.float16
ALU = mybir.AluOpType
ACTF = mybir.ActivationFunctionType

# (i, j, need_min, need_max) per structure; designed + 0/1-verified offline.
SORT5 = [(0, 1, 1, 1), (3, 4, 1, 1), (2, 4, 1, 1), (2, 3, 1, 1), (0, 3, 1, 1),
         (0, 2, 1, 1), (1, 4, 1, 1), (1, 3, 1, 1), (1, 2, 1, 1)]
T_CES = [(0, 5, 1, 1), (4, 9, 1, 1), (4, 5, 1, 1), (2, 7, 1, 1), (2, 4, 1, 1),
         (7, 5, 1, 1), (1, 6, 1, 1), (3, 8, 1, 1), (3, 6, 1, 1), (1, 2, 1, 1),
         (3, 4, 1, 1), (6, 7, 1, 1), (8, 5, 1, 1)]
F_CES = [(0, 10, 0, 1), (5, 15, 1, 0), (5, 10, 1, 1), (4, 14, 1, 1),
         (4, 5, 0, 1), (14, 10, 1, 0), (2, 12, 0, 1), (7, 17, 1, 0),
         (7, 12, 1, 1), (7, 5, 0, 1), (12, 14, 1, 1), (1, 11, 0, 1),
         (9, 19, 1, 0), (9, 11, 1, 1), (6, 16, 1, 1), (6, 9, 0, 1),
         (16, 11, 1, 0), (3, 13, 0, 1), (8, 18, 1, 0), (8, 13, 1, 1),
         (8, 9, 1, 1), (13, 16, 1, 0), (8, 5, 1, 1), (9, 12, 1, 1),
         (13, 14, 1, 1), (8, 20, 0, 1), (13, 24, 1, 0), (13, 20, 0, 1),
         (9, 22, 0, 1), (22, 20, 1, 0), (5, 21, 0, 1), (14, 21, 1, 0),
         (12, 23, 1, 0), (12, 14, 0, 1), (14, 22, 1, 0)]
F_OUT = 14


def _space_ces(ces):
    """Topologically re-sort a CE list so consecutive CEs touch disjoint
    wire positions where possible. Any order that keeps the relative order
    of CEs sharing a position computes the exact same network (independent
    comparators commute), but spaced CEs pipeline better on the DVE: with
    the A/B chunk interleave this lifts producer->consumer distance from 2
    to ~4 ops, hiding the measured ~150ns RAW latency bubble."""
    n = len(ces)
    last_touch = {}
    preds = [set() for _ in range(n)]
    for k, ce in enumerate(ces):
        for p in ce[:2]:
            if p in last_touch:
                preds[k].add(last_touch[p])
            last_touch[p] = k
    indeg = [0] * n
    succs = [[] for _ in range(n)]
    for k, ps in enumerate(preds):
        for q in ps:
            succs[q].append(k)
            indeg[k] += 1
    ready = [k for k in range(n) if indeg[k] == 0]
    out = []
    recent = []
    while ready:
        pick = None
        for k in sorted(ready):
            if not (set(ces[k][:2]) & set(recent)):
                pick = k
                break
        if pick is None:
            pick = sorted(ready)[0]
        ready.remove(pick)
        out.append(ces[pick])
        recent = (list(ces[pick][:2]) + recent)[:4]
        for t in succs[pick]:
            indeg[t] -= 1
            if indeg[t] == 0:
                ready.append(t)
    assert len(out) == n
    return out


SORT5 = _space_ces(SORT5)
T_CES = _space_ces(T_CES)
F_CES = _space_ces(F_CES)

H = 512
W = 512
IMGS_PER_CORE = 2
N_CORES = 8
WIDE = W + 4          # 2-col halo each side
NVB_COLS = 8

# NOTE: the Pool/GPSIMD engine only accepts TensorTensor add/subtract/mult
# and TensorScalar opcodes (no tensor-tensor min/max, no stt, no scan), so
# the median comparator network runs entirely on the DVE; GPS gets the
# pointwise formula tail + the dd combine.
ROWS = 2              # 128-row blocks per SBUF tile (free dim = ROWS*W)
CHUNKS = (H // 128) // ROWS   # chunks per image
NBUF_A = 46           # DVE median pool cap (two interleaved networks)


class BufPool:
    """Free-list over preallocated fixed SBUF tensors. Tile's dependency
    tracker makes reuse safe (WAR/RAW serialization on the same tensor)."""

    def __init__(self, nc, prefix, width, cap):
        self.nc = nc
        self.prefix = prefix
        self.width = width
        self.cap = cap
        self.bufs = []
        self.free = []
        self.peak = 0

    def alloc(self):
        # Prefer a fresh tensor until the cap is reached, then take the
        # OLDEST released one (FIFO): with the DVE deeply pipelined, a
        # LIFO free-list rewrites a buffer whose reader retired only 1-2
        # ops ago (distance-1 WAR); rotating through cap-many buffers
        # pushes the write-after-read distance to ~(cap - live) ops.
        idx = len(self.bufs)
        if idx < self.cap:
            t = self.nc.alloc_sbuf_tensor(
                f"{self.prefix}{idx}", [128, ROWS, self.width], F16).ap()
            self.bufs.append(t)
            self.peak = max(self.peak, idx + 1)
            return t
        return self.free.pop(0)

    def release(self, t):
        self.free.append(t)


class Wire:
    """SSA value living at column offset `off` of `buf`."""

    def __init__(self, buf, off, owned, pool, on_die=None):
        self.buf = buf
        self.off = off
        self.owned = owned      # release buf to pool when dead
        self.pool = pool
        self.reads_left = 0
        self.on_die = on_die

    def ap(self, width):
        return self.buf[:, :, self.off:self.off + width]

    def read_done(self):
        self.reads_left -= 1
        if self.reads_left == 0:
            self._die()

    def read_done_zero(self):
        if self.reads_left == 0:
            self._die()

    def _die(self):
        if self.owned:
            self.pool.release(self.buf)
        if self.on_die is not None:
            self.on_die()

    def detach_views(self, n_views):
        """Transfer buffer ownership to n_views future views; returns the
        on_die callback the views share. Call read_done() after to consume
        the terminal hold."""
        buf, owned, pool = self.buf, self.owned, self.pool
        self.owned = False
        state = {"n": n_views}

        def on_die():
            state["n"] -= 1
            if state["n"] == 0 and owned:
                pool.release(buf)
        return on_die


def run_stage(eng, pool, wires, ces, width, terminal_reads):
    """Emit one structure stage on engine `eng`. A position's lifetime is
    split into segments at each rewrite; each Wire object gets the read count
    of its own segment so buffers release as soon as truly dead."""
    n = len(wires)
    segs = [[] for _ in range(n)]
    cur = [0] * n
    for (a, b, nmin, nmax) in ces:
        cur[a] += 1
        cur[b] += 1
        if nmin:
            segs[a].append(cur[a])
            cur[a] = 0
        if nmax:
            segs[b].append(cur[b])
            cur[b] = 0
    for i in range(n):
        segs[i].append(cur[i] + terminal_reads.get(i, 0))

    seg_idx = [0] * n
    for i in range(n):
        wires[i].reads_left += segs[i][0]
        if segs[i][0] == 0:
            wires[i].read_done_zero()

    for (i, j, nmin, nmax) in ces:
        wi, wj = wires[i], wires[j]
        a = wi.ap(width)
        b = wj.ap(width)
        if nmin:
            lo = pool.alloc()
            eng.tensor_tensor(lo[:, :, 0:width], a, b, ALU.min)
            yield
        if nmax:
            hi = pool.alloc()
            eng.tensor_tensor(hi[:, :, 0:width], a, b, ALU.max)
            yield
        wi.read_done()
        wj.read_done()
        if nmin:
            seg_idx[i] += 1
            cnt = segs[i][seg_idx[i]]
            assert cnt > 0, "dead write (should be pruned offline)"
            wires[i] = Wire(lo, 0, True, pool)
            wires[i].reads_left = cnt
        if nmax:
            seg_idx[j] += 1
            cnt = segs[j][seg_idx[j]]
            assert cnt > 0, "dead write (should be pruned offline)"
            wires[j] = Wire(hi, 0, True, pool)
            wires[j].reads_left = cnt


def emit_median(eng, pool, tin, lo, fw, sw, tw, out):
    """Generator: emits the fp16 median network for output columns
    [lo, lo+fw) on engine `eng`, yielding after every op so two chunks'
    networks can be interleaved (hides the DVE RAW latency bubble,
    measured ~18%% on HW). The median Wire lands in out[0]."""
    s_wires = [Wire(tin[k], lo, False, pool) for k in range(5)]
    yield from run_stage(eng, pool, s_wires, SORT5, sw, {k: 1 for k in range(5)})

    t_wires = [None] * 10
    c_views = [None] * 5
    for k in range(5):
        rk = s_wires[k]
        od = rk.detach_views(3)
        t_wires[k] = Wire(rk.buf, rk.off + 0, False, pool, on_die=od)
        t_wires[k + 5] = Wire(rk.buf, rk.off + 1, False, pool, on_die=od)
        c_views[k] = Wire(rk.buf, rk.off + 4, False, pool, on_die=od)
        rk.read_done()      # consume terminal hold

    yield from run_stage(eng, pool, t_wires, T_CES, tw, {j: 1 for j in range(10)})

    f_wires = [None] * 25
    for j in range(10):
        twr = t_wires[j]
        od = twr.detach_views(2)
        f_wires[j] = Wire(twr.buf, twr.off + 0, False, pool, on_die=od)
        f_wires[j + 10] = Wire(twr.buf, twr.off + 2, False, pool, on_die=od)
        twr.read_done()
    for k in range(5):
        f_wires[20 + k] = c_views[k]

    yield from run_stage(eng, pool, f_wires, F_CES, fw, {F_OUT: 1})
    out[0] = f_wires[F_OUT]


def emit_formula(nc, pool, tin, fw, mid, t1, g16, nb_ap, out_tile):
    """Generator: y = relu(x - g*((x + nb) - mid)), all fp16 on the DVE
    fast path. (Measured on HW: Pool-engine elementwise ops are far slower
    than the cost model claims and serialize the g-chain, so keep
    everything on the DVE.)"""
    xc = tin[2][:, :, 2:2 + fw]                 # center plane = x (fp16)
    dve = nc.vector
    dve.scalar_tensor_tensor(t1[:, :, 0:fw], xc, nb_ap, mid.ap(fw),
                             ALU.add, ALU.subtract)
    mid.read_done()
    yield
    dve.tensor_tensor(t1[:, :, 0:fw], t1[:, :, 0:fw],
                      g16[:, :, 0:fw], ALU.mult)
    yield
    dve.tensor_tensor(t1[:, :, 0:fw], xc, t1[:, :, 0:fw], ALU.subtract)
    yield
    dve.tensor_scalar(out_tile[:, :, 0:fw], t1[:, :, 0:fw], 0.0, None, ALU.max)
    yield


def build_module(repeat=1, hw_loop=None):
    nc = bacc.Bacc(
        "TRN2",
        target_bir_lowering=False,
        debug=False,
        enable_asserts=False,
        num_devices=N_CORES,
    )
    x = nc.dram_tensor("x", [IMGS_PER_CORE, H + 4, WIDE], F16,
                       kind="ExternalInput")
    nvb = nc.dram_tensor("nvb", [128, NVB_COLS], F32, kind="ExternalInput")
    bs = nc.dram_tensor("bs", [128, 128], F16, kind="ExternalInput")
    bst = nc.dram_tensor("bst", [4, 128], F16, kind="ExternalInput")
    bq = nc.dram_tensor("bq", [128, 128], F32, kind="ExternalInput")
    bqt = nc.dram_tensor("bqt", [4, 128], F32, kind="ExternalInput")
    y = nc.dram_tensor("y", [IMGS_PER_CORE, H, W], F16, kind="ExternalOutput")

    xa = x.ap().flatten_outer_dims()    # [2*516, 516] fp16
    ya = y.ap().flatten_outer_dims()

    with tile.TileContext(nc) as tc:
        poolA = BufPool(nc, "pa", WIDE, NBUF_A)

        nvb_t = nc.alloc_sbuf_tensor("nvb_t", [128, NVB_COLS], F32).ap()
        nc.sync.dma_start(nvb_t[:, :], nvb.ap()[:, :])
        nb_ap = nvb_t[:, 0:1]
        c2_ap = nvb_t[:, 1:2]
        sqlam_ap = nvb_t[:, 2:3]
        epsp_ap = nvb_t[:, 3:4]

        bs_t = nc.alloc_sbuf_tensor("bs_t", [128, 128], F16).ap()
        bst_t = nc.alloc_sbuf_tensor("bst_t", [4, 128], F16).ap()
        bq_t = nc.alloc_sbuf_tensor("bq_t", [128, 128], F32).ap()
        bqt_t = nc.alloc_sbuf_tensor("bqt_t", [4, 128], F32).ap()
        for src, dst in ((bs, bs_t), (bst, bst_t), (bq, bq_t), (bqt, bqt_t)):
            nc.sync.dma_start(dst[:, :], src.ap()[:, :])

        # double-buffered input tiles (fp16) + output tiles
        tins = [[nc.alloc_sbuf_tensor(f"tin{p}_{k}", [128, ROWS, WIDE], F16).ap()
                 for k in range(5)] for p in range(2)]
        # 4 tail rows per block for the PE contraction (must start at
        # partition 0: matmul operands need base partition 0/32/64)
        tints = [nc.alloc_sbuf_tensor(f"tint{p}", [4, ROWS, WIDE], F16).ap()
                 for p in range(2)]
        outs = [nc.alloc_sbuf_tensor(f"oa{p}", [128, ROWS, W], F16).ap()
                for p in range(2)]

        # variance-path tensors, one set per parity so an interleaved chunk
        # pair never serializes on them
        def var_set(p):
            return dict(
                t1b=nc.alloc_sbuf_tensor(f"t1b{p}", [128, ROWS, W], F16).ap(),
                sqm=nc.alloc_sbuf_tensor(f"sqm{p}", [128, ROWS, WIDE], F32).ap(),
                sqt=nc.alloc_sbuf_tensor(f"sqt{p}", [4, ROWS, WIDE], F32).ap(),
                svb=nc.alloc_sbuf_tensor(f"svb{p}", [128, ROWS, WIDE + 5], F32).ap(),
                qvb=nc.alloc_sbuf_tensor(f"qvb{p}", [128, ROWS, WIDE + 5], F32).ap(),
                s25=nc.alloc_sbuf_tensor(f"s25{p}", [128, ROWS, WIDE], F32).ap(),
                q25=nc.alloc_sbuf_tensor(f"q25{p}", [128, ROWS, WIDE], F32).ap(),
                s2b=nc.alloc_sbuf_tensor(f"s2b{p}", [128, ROWS, W], F32).ap(),
                ddb=nc.alloc_sbuf_tensor(f"ddb{p}", [128, ROWS, W], F32).ap(),
                gb=nc.alloc_sbuf_tensor(f"gb{p}", [128, ROWS, W], F32).ap(),
                g16=nc.alloc_sbuf_tensor(f"g16{p}", [128, ROWS, W], F16).ap(),
            )
        vsets = [var_set(p) for p in range(2)]

        # zero the 5 leading halo cols of the scan buffers (persist: the
        # per-chunk writes only touch cols [5, 521))
        for v in vsets:
            nc.vector.memset(v["svb"][:, :, 0:5], 0.0)
            nc.vector.memset(v["qvb"][:, :, 0:5], 0.0)

        # PSUM: 4 big banks + 1 shared tail bank (shared across parities;
        # the PE->copy sequences of a pair are emitted head-A then head-B,
        # so bank reuse serializes only the cheap PE/ACT head work)
        ps = [nc.alloc_psum_tensor(f"ps{b}", [128, W], F32).ap()
              for b in range(2)]
        pq = [nc.alloc_psum_tensor(f"pq{b}", [128, W], F32).ap()
              for b in range(2)]
        pt = nc.alloc_psum_tensor("pt", [128, 8 * ROWS], F32).ap()

        def emit_head(img, ci, par):
            """Loads + squares + PE vertical sums + PSUM->SBUF copies."""
            tin = tins[par]
            tint = tints[par]
            v = vsets[par]
            r0 = ci * 128 * ROWS

            for k, dy in enumerate(range(-2, 3)):
                for b in range(ROWS):
                    s = img * (H + 4) + r0 + b * 128 + dy + 2
                    nc.sync.dma_start(tin[k][:, b, :], xa[s: s + 128, :])
            for b in range(ROWS):
                s = img * (H + 4) + r0 + b * 128 + 128
                nc.sync.dma_start(tint[:, b, :], xa[s: s + 4, :])

            nc.scalar.square(v["sqm"][:, :, :], tin[0][:, :, :])
            nc.scalar.square(v["sqt"][:, :, :], tint[:, :, :])

            for b in range(ROWS):
                for (P, tcol, wmain, wtail, rm, rt) in (
                    (ps[b % 2], 8 * b + 0, bs_t, bst_t,
                     tin[0][:, b, :], tint[:, b, :]),
                    (pq[b % 2], 8 * b + 4, bq_t, bqt_t,
                     v["sqm"][:, b, :], v["sqt"][:, b, :]),
                ):
                    nc.tensor.matmul(P[:, 0:W], wmain, rm[:, 0:W],
                                     start=True, stop=False)
                    nc.tensor.matmul(P[:, 0:W], wtail, rt[:, 0:W],
                                     start=False, stop=True)
                    nc.tensor.matmul(pt[:, tcol:tcol + 4], wmain, rm[:, W:WIDE],
                                     start=True, stop=False)
                    nc.tensor.matmul(pt[:, tcol:tcol + 4], wtail, rt[:, W:WIDE],
                                     start=False, stop=True)
                nc.scalar.copy(v["svb"][:, b, 5:5 + W], ps[b % 2][:, :])
                nc.scalar.copy(v["qvb"][:, b, 5:5 + W], pq[b % 2][:, :])
                nc.scalar.copy(v["svb"][:, b, 5 + W:5 + WIDE],
                               pt[:, 8 * b:8 * b + 4])
                nc.scalar.copy(v["qvb"][:, b, 5 + W:5 + WIDE],
                               pt[:, 8 * b + 4:8 * b + 8])

        def chunk_gen(img, ci, par):
            """Generator emitting this chunk's DVE stream (median, scans,
            dd, reciprocal, formula) + its ACT tail + stores, yielding after
            every DVE op for pairwise interleaving."""
            tin = tins[par]
            out_t = outs[par]
            v = vsets[par]
            r0 = ci * 128 * ROWS

            mid_box = [None]
            yield from emit_median(nc.vector, poolA, tin, 0, W, WIDE, W + 3,
                                   mid_box)

            for b in range(ROWS):
                nc.vector.tensor_tensor_scan(
                    v["s25"][:, b, :], v["svb"][:, b, 5:5 + WIDE],
                    v["svb"][:, b, 0:WIDE], 0.0, ALU.add, ALU.subtract)
                yield
                nc.vector.tensor_tensor_scan(
                    v["q25"][:, b, :], v["qvb"][:, b, 5:5 + WIDE],
                    v["qvb"][:, b, 0:WIDE], epsp_ap, ALU.add, ALU.subtract)
                yield
            # s2 = (sqlam * s25)^2 on ACT; dd = c2*q25 - s2; g = 1/dd
            nc.scalar.activation(v["s2b"][:, :, :], v["s25"][:, :, 4:4 + W],
                                 ACTF.Square, 0.0, sqlam_ap)
            nc.vector.scalar_tensor_tensor(v["ddb"][:, :, :],
                                           v["q25"][:, :, 4:4 + W],
                                           c2_ap, v["s2b"][:, :, :],
                                           ALU.mult, ALU.subtract)
            yield
            nc.vector.reciprocal_approx_fast(out=v["gb"][:, :, :],
                                             in_=v["ddb"][:, :, :])
            yield
            nc.scalar.copy(v["g16"][:, :, :], v["gb"][:, :, :])

            yield from emit_formula(nc, poolA, tin, W, mid_box[0], v["t1b"],
                                    v["g16"], nb_ap, out_t)

            for b in range(ROWS):
                row = img * H + r0 + b * 128
                nc.sync.dma_start(ya[row:row + 128, :], out_t[:, b, :])

        def body():
            for _ in range(repeat):
                for img in range(IMGS_PER_CORE):
                    # chunk pair (the two halves of one image), heads first,
                    # then the DVE streams interleaved op-by-op
                    emit_head(img, 0, 0)
                    emit_head(img, 1, 1)
                    ga = chunk_gen(img, 0, 0)
                    gc = chunk_gen(img, 1, 1)
                    done_a = done_b = False
                    while not (done_a and done_b):
                        if not done_a:
                            try:
                                next(ga)
                            except StopIteration:
                                done_a = True
                        if not done_b:
                            try:
                                next(gc)
                            except StopIteration:
                                done_b = True

        if hw_loop is None:
            body()
        else:
            with tc.For_i(0, hw_loop, 1):
                body()

    global _PEAKS
    _PEAKS = (poolA.peak,)
    nc.compile()
    return nc


_PEAKS = None


_MODULE = None


def _get_module():
    global _MODULE
    if _MODULE is None:
        _MODULE = build_module()
    return _MODULE


def prepare_in_maps(x, noise_var, noise_bias):
    """Host-side prep: fp16 padded shards + banded PE weights + scalars."""
    x = np.ascontiguousarray(np.asarray(x, dtype=np.float32))
    nv = float(np.asarray(noise_var).reshape(-1)[0])
    nb = float(np.asarray(noise_bias).reshape(-1)[0])
    B = x.shape[0]
    assert x.shape == (B, 1, H, W) and B == N_CORES * IMGS_PER_CORE

    xpad = np.zeros((B, H + 4, WIDE), np.float16)
    xpad[:, 2:2 + H, 2:2 + W] = x[:, 0]

    w1h = np.float16(1.0 / np.sqrt(600.0 * nv))
    w1 = float(w1h)
    w2 = float(np.float32(1.0 / (24.0 * nv)))
    k_idx = np.arange(128)[:, None]
    m_idx = np.arange(128)[None, :]
    band = (np.abs(k_idx - 2 - m_idx) <= 2)
    bandt = (np.abs(128 + np.arange(4)[:, None] - 2 - m_idx) <= 2)
    bs = (band * w1).astype(np.float16)
    bst = (bandt * w1).astype(np.float16)
    bq = (band * w2).astype(np.float32)
    bqt = (bandt * w2).astype(np.float32)

    c2 = 1.0 / (24.0 * nv * w2)
    sqlam = np.sqrt(1.0 / (600.0 * nv * w1 * w1))
    eps_p = (1e-10 / nv) / c2
    nvb = np.zeros((128, NVB_COLS), np.float32)
    nvb[:, 0] = nb
    nvb[:, 1] = c2
    nvb[:, 2] = sqlam
    nvb[:, 3] = eps_p

    in_maps = []
    for c in range(N_CORES):
        shard = np.ascontiguousarray(
            xpad[c * IMGS_PER_CORE:(c + 1) * IMGS_PER_CORE])
        in_maps.append({"x": shard, "nvb": nvb, "bs": bs, "bst": bst,
                        "bq": bq, "bqt": bqt})
    return in_maps


def kernel(x, noise_var, noise_bias):
    in_maps = prepare_in_maps(x, noise_var, noise_bias)
    B = N_CORES * IMGS_PER_CORE
    nc = _get_module()
    res = run_bass_kernel_spmd(nc, in_maps, core_ids=list(range(N_CORES)))
    y = np.empty((B, 1, H, W), np.float32)
    for c in range(N_CORES):
        y[c * IMGS_PER_CORE:(c + 1) * IMGS_PER_CORE, 0] = \
            res.results[c]["y"].astype(np.float32)
    return y


# revision 40
# speedup vs baseline: 1.8665x; 1.8665x over previous
"""Trainium2 Bass kernel: 5x5 window median+variance denoise filter.

y = relu(x - noise_var/(var5x5(x)+1e-10) * (x - median5x5(x) + noise_bias))
with zero-padded 5x5 windows, unbiased variance (ddof=1).

Sharding: pure data parallel, B=16 images split 2-per-core across 8 cores.

Design (v1 was an fp32 DVE-only comparator network at 668us/core):
  * the median comparator network runs in fp16 on the DVE; the input is
    pre-rounded to fp16 on the host, so tiles are fp16 end-to-end and every
    tensor_tensor min/max gets the DVE 2-byte fast path. Measured on HW:
    [128, 2, 516] fp16 ops run ~1.7x faster than fp32 (the [128, 4, 516]
    shape falls back to 1x - keep ROWS=2). Accuracy verified offline:
    full-fp16 pipeline rel err 1.3e-3 vs the 2e-2 gate (the error
    amplification factor nv/(var+eps) maxes at ~10 on this input).
  * variance path off the DVE almost entirely: vertical 5-row sums via PE
    matmuls against banded weight matrices (host-built, nv folded into the
    weights; fp16 weights for the x-plane, fp32 for the x^2-plane; exact
    fp32 PSUM accumulation); horizontal sliding 5-sums via single DVE
    tensor_tensor_scan ops (state = (Sv[c] + state) - Sv[c-5]); squares,
    PSUM->SBUF copies and fp16 downconverts on the Activation engine.
    All rescaling constants (c2, sqrt(lam), eps') are folded host-side so
    the math matches the reference exactly up to rounding.
  * the two chunks (halves) of an image are emitted as op-interleaved
    DVE streams, and each stage's comparator list is topologically
    re-sorted so consecutive CEs touch disjoint positions (_space_ces).
    Together these give producer->consumer distances >= 4, which lets the
    DVE pipeline ops back-to-back: measured 287us (serial) -> 248us
    (pairwise interleave) -> 144us (interleave + CE spacing). Each parity
    has its own input tiles and variance buffers so the pair never
    serializes. (Measured dead end: widening the buffer pool to 46 with
    FIFO rotation REGRESSED to 269us - keep the tight LIFO free-list.)
  * reciprocal via the single-op DVE reciprocal_approx_fast; formula tail
    in fp16 on the DVE. (Measured dead ends: the Pool/GPSIMD engine only
    accepts TensorTensor add/sub/mult + TensorScalar opcodes -- no
    tensor-tensor min/max, so the median cannot be split onto it -- and
    its real elementwise throughput is ~10x below the cost model, so even
    the formula tail is faster on the DVE. ROWS=4 tiles drop the DVE
    fp16 fast path to 1x; keep ROWS=2.)
  * fp16 output store, upcast to fp32 on the host.
"""
import numpy as np

import concourse.bass as bass  # noqa: F401
import concourse.mybir as mybir
from concourse import bacc, tile
from concourse.bass_utils import run_bass_kernel_spmd

F32 = mybir.dt.float32
F16 = mybir.dt.float16
ALU = mybir.AluOpType
ACTF = mybir.ActivationFunctionType

# (i, j, need_min, need_max) per structure; designed + 0/1-verified offline.
SORT5 = [(0, 1, 1, 1), (3, 4, 1, 1), (2, 4, 1, 1), (2, 3, 1, 1), (0, 3, 1, 1),
         (0, 2, 1, 1), (1, 4, 1, 1), (1, 3, 1, 1), (1, 2, 1, 1)]
T_CES = [(0, 5, 1, 1), (4, 9, 1, 1), (4, 5, 1, 1), (2, 7, 1, 1), (2, 4, 1, 1),
         (7, 5, 1, 1), (1, 6, 1, 1), (3, 8, 1, 1), (3, 6, 1, 1), (1, 2, 1, 1),
         (3, 4, 1, 1), (6, 7, 1, 1), (8, 5, 1, 1)]
F_CES = [(0, 10, 0, 1), (5, 15, 1, 0), (5, 10, 1, 1), (4, 14, 1, 1),
         (4, 5, 0, 1), (14, 10, 1, 0), (2, 12, 0, 1), (7, 17, 1, 0),
         (7, 12, 1, 1), (7, 5, 0, 1), (12, 14, 1, 1), (1, 11, 0, 1),
         (9, 19, 1, 0), (9, 11, 1, 1), (6, 16, 1, 1), (6, 9, 0, 1),
         (16, 11, 1, 0), (3, 13, 0, 1), (8, 18, 1, 0), (8, 13, 1, 1),
         (8, 9, 1, 1), (13, 16, 1, 0), (8, 5, 1, 1), (9, 12, 1, 1),
         (13, 14, 1, 1), (8, 20, 0, 1), (13, 24, 1, 0), (13, 20, 0, 1),
         (9, 22, 0, 1), (22, 20, 1, 0), (5, 21, 0, 1), (14, 21, 1, 0),
         (12, 23, 1, 0), (12, 14, 0, 1), (14, 22, 1, 0)]
F_OUT = 14


def _space_ces(ces):
    """Topologically re-sort a CE list so consecutive CEs touch disjoint
    wire positions where possible. Any order that keeps the relative order
    of CEs sharing a position computes the exact same network (independent
    comparators commute), but spaced CEs pipeline better on the DVE: with
    the A/B chunk interleave this lifts producer->consumer distance from 2
    to ~4 ops, hiding the measured ~150ns RAW latency bubble."""
    n = len(ces)
    last_touch = {}
    preds = [set() for _ in range(n)]
    for k, ce in enumerate(ces):
        for p in ce[:2]:
            if p in last_touch:
                preds[k].add(last_touch[p])
            last_touch[p] = k
    indeg = [0] * n
    succs = [[] for _ in range(n)]
    for k, ps in enumerate(preds):
        for q in ps:
            succs[q].append(k)
            indeg[k] += 1
    ready = [k for k in range(n) if indeg[k] == 0]
    out = []
    recent = []
    while ready:
        pick = None
        for k in sorted(ready):
            if not (set(ces[k][:2]) & set(recent)):
                pick = k
                break
        if pick is None:
            pick = sorted(ready)[0]
        ready.remove(pick)
        out.append(ces[pick])
        recent = (list(ces[pick][:2]) + recent)[:4]
        for t in succs[pick]:
            indeg[t] -= 1
            if indeg[t] == 0:
                ready.append(t)
    assert len(out) == n
    return out


SORT5 = _space_ces(SORT5)
T_CES = _space_ces(T_CES)
F_CES = _space_ces(F_CES)

H = 512
W = 512
IMGS_PER_CORE = 2
N_CORES = 8
WIDE = W + 4          # 2-col halo each side
NVB_COLS = 8

# NOTE: the Pool/GPSIMD engine only accepts TensorTensor add/subtract/mult
# and TensorScalar opcodes (no tensor-tensor min/max, no stt, no scan), so
# the median comparator network runs entirely on the DVE; GPS gets the
# pointwise formula tail + the dd combine.
ROWS = 2              # 128-row blocks per SBUF tile (free dim = ROWS*W)
CHUNKS = (H // 128) // ROWS   # chunks per image
NBUF_A = 40           # DVE median pool cap (two interleaved networks)


class BufPool:
    """Free-list over preallocated fixed SBUF tensors. Tile's dependency
    tracker makes reuse safe (WAR/RAW serialization on the same tensor)."""

    def __init__(self, nc, prefix, width, cap):
        self.nc = nc
        self.prefix = prefix
        self.width = width
        self.cap = cap
        self.bufs = []
        self.free = []
        self.peak = 0

    def alloc(self):
        if self.free:
            return self.free.pop()
        idx = len(self.bufs)
        assert idx < self.cap, f"SBUF pool {self.prefix} exhausted"
        t = self.nc.alloc_sbuf_tensor(
            f"{self.prefix}{idx}", [128, ROWS, self.width], F16).ap()
        self.bufs.append(t)
        self.peak = max(self.peak, idx + 1)
        return t

    def release(self, t):
        self.free.append(t)


class Wire:
    """SSA value living at column offset `off` of `buf`."""

    def __init__(self, buf, off, owned, pool, on_die=None):
        self.buf = buf
        self.off = off
        self.owned = owned      # release buf to pool when dead
        self.pool = pool
        self.reads_left = 0
        self.on_die = on_die

    def ap(self, width):
        return self.buf[:, :, self.off:self.off + width]

    def read_done(self):
        self.reads_left -= 1
        if self.reads_left == 0:
            self._die()

    def read_done_zero(self):
        if self.reads_left == 0:
            self._die()

    def _die(self):
        if self.owned:
            self.pool.release(self.buf)
        if self.on_die is not None:
            self.on_die()

    def detach_views(self, n_views):
        """Transfer buffer ownership to n_views future views; returns the
        on_die callback the views share. Call read_done() after to consume
        the terminal hold."""
        buf, owned, pool = self.buf, self.owned, self.pool
        self.owned = False
        state = {"n": n_views}

        def on_die():
            state["n"] -= 1
            if state["n"] == 0 and owned:
                pool.release(buf)
        return on_die


def run_stage(eng, pool, wires, ces, width, terminal_reads):
    """Emit one structure stage on engine `eng`. A position's lifetime is
    split into segments at each rewrite; each Wire object gets the read count
    of its own segment so buffers release as soon as truly dead."""
    n = len(wires)
    segs = [[] for _ in range(n)]
    cur = [0] * n
    for (a, b, nmin, nmax) in ces:
        cur[a] += 1
        cur[b] += 1
        if nmin:
            segs[a].append(cur[a])
            cur[a] = 0
        if nmax:
            segs[b].append(cur[b])
            cur[b] = 0
    for i in range(n):
        segs[i].append(cur[i] + terminal_reads.get(i, 0))

    seg_idx = [0] * n
    for i in range(n):
        wires[i].reads_left += segs[i][0]
        if segs[i][0] == 0:
            wires[i].read_done_zero()

    for (i, j, nmin, nmax) in ces:
        wi, wj = wires[i], wires[j]
        a = wi.ap(width)
        b = wj.ap(width)
        if nmin:
            lo = pool.alloc()
            eng.tensor_tensor(lo[:, :, 0:width], a, b, ALU.min)
            yield
        if nmax:
            hi = pool.alloc()
            eng.tensor_tensor(hi[:, :, 0:width], a, b, ALU.max)
            yield
        wi.read_done()
        wj.read_done()
        if nmin:
            seg_idx[i] += 1
            cnt = segs[i][seg_idx[i]]
            assert cnt > 0, "dead write (should be pruned offline)"
            wires[i] = Wire(lo, 0, True, pool)
            wires[i].reads_left = cnt
        if nmax:
            seg_idx[j] += 1
            cnt = segs[j][seg_idx[j]]
            assert cnt > 0, "dead write (should be pruned offline)"
            wires[j] = Wire(hi, 0, True, pool)
            wires[j].reads_left = cnt


def emit_median(eng, pool, tin, lo, fw, sw, tw, out):
    """Generator: emits the fp16 median network for output columns
    [lo, lo+fw) on engine `eng`, yielding after every op so two chunks'
    networks can be interleaved (hides the DVE RAW latency bubble,
    measured ~18%% on HW). The median Wire lands in out[0]."""
    s_wires = [Wire(tin[k], lo, False, pool) for k in range(5)]
    yield from run_stage(eng, pool, s_wires, SORT5, sw, {k: 1 for k in range(5)})

    t_wires = [None] * 10
    c_views = [None] * 5
    for k in range(5):
        rk = s_wires[k]
        od = rk.detach_views(3)
        t_wires[k] = Wire(rk.buf, rk.off + 0, False, pool, on_die=od)
        t_wires[k + 5] = Wire(rk.buf, rk.off + 1, False, pool, on_die=od)
        c_views[k] = Wire(rk.buf, rk.off + 4, False, pool, on_die=od)
        rk.read_done()      # consume terminal hold

    yield from run_stage(eng, pool, t_wires, T_CES, tw, {j: 1 for j in range(10)})

    f_wires = [None] * 25
    for j in range(10):
        twr = t_wires[j]
        od = twr.detach_views(2)
        f_wires[j] = Wire(twr.buf, twr.off + 0, False, pool, on_die=od)
        f_wires[j + 10] = Wire(twr.buf, twr.off + 2, False, pool, on_die=od)
        twr.read_done()
    for k in range(5):
        f_wires[20 + k] = c_views[k]

    yield from run_stage(eng, pool, f_wires, F_CES, fw, {F_OUT: 1})
    out[0] = f_wires[F_OUT]


def emit_formula(nc, pool, tin, fw, mid, t1, g16, nb_ap, out_tile):
    """Generator: y = relu(x - g*((x + nb) - mid)), all fp16 on the DVE
    fast path. (Measured on HW: Pool-engine elementwise ops are far slower
    than the cost model claims and serialize the g-chain, so keep
    everything on the DVE.)"""
    xc = tin[2][:, :, 2:2 + fw]                 # center plane = x (fp16)
    dve = nc.vector
    dve.scalar_tensor_tensor(t1[:, :, 0:fw], xc, nb_ap, mid.ap(fw),
                             ALU.add, ALU.subtract)
    mid.read_done()
    yield
    dve.tensor_tensor(t1[:, :, 0:fw], t1[:, :, 0:fw],
                      g16[:, :, 0:fw], ALU.mult)
    yield
    dve.tensor_tensor(t1[:, :, 0:fw], xc, t1[:, :, 0:fw], ALU.subtract)
    yield
    dve.tensor_scalar(out_tile[:, :, 0:fw], t1[:, :, 0:fw], 0.0, None, ALU.max)
    yield


def build_module(repeat=1, hw_loop=None):
    nc = bacc.Bacc(
        "TRN2",
        target_bir_lowering=False,
        debug=False,
        enable_asserts=False,
        num_devices=N_CORES,
    )
    x = nc.dram_tensor("x", [IMGS_PER_CORE, H + 4, WIDE], F16,
                       kind="ExternalInput")
    nvb = nc.dram_tensor("nvb", [128, NVB_COLS], F32, kind="ExternalInput")
    bs = nc.dram_tensor("bs", [128, 128], F16, kind="ExternalInput")
    bst = nc.dram_tensor("bst", [4, 128], F16, kind="ExternalInput")
    bq = nc.dram_tensor("bq", [128, 128], F32, kind="ExternalInput")
    bqt = nc.dram_tensor("bqt", [4, 128], F32, kind="ExternalInput")
    y = nc.dram_tensor("y", [IMGS_PER_CORE, H, W], F16, kind="ExternalOutput")

    xa = x.ap().flatten_outer_dims()    # [2*516, 516] fp16
    ya = y.ap().flatten_outer_dims()

    with tile.TileContext(nc) as tc:
        poolA = BufPool(nc, "pa", WIDE, NBUF_A)

        nvb_t = nc.alloc_sbuf_tensor("nvb_t", [128, NVB_COLS], F32).ap()
        nc.sync.dma_start(nvb_t[:, :], nvb.ap()[:, :])
        nb_ap = nvb_t[:, 0:1]
        c2_ap = nvb_t[:, 1:2]
        sqlam_ap = nvb_t[:, 2:3]
        epsp_ap = nvb_t[:, 3:4]

        bs_t = nc.alloc_sbuf_tensor("bs_t", [128, 128], F16).ap()
        bst_t = nc.alloc_sbuf_tensor("bst_t", [4, 128], F16).ap()
        bq_t = nc.alloc_sbuf_tensor("bq_t", [128, 128], F32).ap()
        bqt_t = nc.alloc_sbuf_tensor("bqt_t", [4, 128], F32).ap()
        for src, dst in ((bs, bs_t), (bst, bst_t), (bq, bq_t), (bqt, bqt_t)):
            nc.sync.dma_start(dst[:, :], src.ap()[:, :])

        # double-buffered input tiles (fp16) + output tiles
        tins = [[nc.alloc_sbuf_tensor(f"tin{p}_{k}", [128, ROWS, WIDE], F16).ap()
                 for k in range(5)] for p in range(2)]
        # 4 tail rows per block for the PE contraction (must start at
        # partition 0: matmul operands need base partition 0/32/64)
        tints = [nc.alloc_sbuf_tensor(f"tint{p}", [4, ROWS, WIDE], F16).ap()
                 for p in range(2)]
        outs = [nc.alloc_sbuf_tensor(f"oa{p}", [128, ROWS, W], F16).ap()
                for p in range(2)]

        # variance-path tensors, one set per parity so an interleaved chunk
        # pair never serializes on them
        def var_set(p):
            return dict(
                t1b=nc.alloc_sbuf_tensor(f"t1b{p}", [128, ROWS, W], F16).ap(),
                sqm=nc.alloc_sbuf_tensor(f"sqm{p}", [128, ROWS, WIDE], F32).ap(),
                sqt=nc.alloc_sbuf_tensor(f"sqt{p}", [4, ROWS, WIDE], F32).ap(),
                svb=nc.alloc_sbuf_tensor(f"svb{p}", [128, ROWS, WIDE + 5], F32).ap(),
                qvb=nc.alloc_sbuf_tensor(f"qvb{p}", [128, ROWS, WIDE + 5], F32).ap(),
                s25=nc.alloc_sbuf_tensor(f"s25{p}", [128, ROWS, WIDE], F32).ap(),
                q25=nc.alloc_sbuf_tensor(f"q25{p}", [128, ROWS, WIDE], F32).ap(),
                s2b=nc.alloc_sbuf_tensor(f"s2b{p}", [128, ROWS, W], F32).ap(),
                ddb=nc.alloc_sbuf_tensor(f"ddb{p}", [128, ROWS, W], F32).ap(),
                gb=nc.alloc_sbuf_tensor(f"gb{p}", [128, ROWS, W], F32).ap(),
                g16=nc.alloc_sbuf_tensor(f"g16{p}", [128, ROWS, W], F16).ap(),
            )
        vsets = [var_set(p) for p in range(2)]

        # zero the 5 leading halo cols of the scan buffers (persist: the
        # per-chunk writes only touch cols [5, 521))
        for v in vsets:
            nc.vector.memset(v["svb"][:, :, 0:5], 0.0)
            nc.vector.memset(v["qvb"][:, :, 0:5], 0.0)

        # PSUM: 4 big banks + 1 shared tail bank (shared across parities;
        # the PE->copy sequences of a pair are emitted head-A then head-B,
        # so bank reuse serializes only the cheap PE/ACT head work)
        ps = [nc.alloc_psum_tensor(f"ps{b}", [128, W], F32).ap()
              for b in range(2)]
        pq = [nc.alloc_psum_tensor(f"pq{b}", [128, W], F32).ap()
              for b in range(2)]
        pt = nc.alloc_psum_tensor("pt", [128, 8 * ROWS], F32).ap()

        def emit_head(img, ci, par):
            """Loads + squares + PE vertical sums + PSUM->SBUF copies."""
            tin = tins[par]
            tint = tints[par]
            v = vsets[par]
            r0 = ci * 128 * ROWS

            for k, dy in enumerate(range(-2, 3)):
                for b in range(ROWS):
                    s = img * (H + 4) + r0 + b * 128 + dy + 2
                    nc.sync.dma_start(tin[k][:, b, :], xa[s: s + 128, :])
            for b in range(ROWS):
                s = img * (H + 4) + r0 + b * 128 + 128
                nc.sync.dma_start(tint[:, b, :], xa[s: s + 4, :])

            nc.scalar.square(v["sqm"][:, :, :], tin[0][:, :, :])
            nc.scalar.square(v["sqt"][:, :, :], tint[:, :, :])

            for b in range(ROWS):
                for (P, tcol, wmain, wtail, rm, rt) in (
                    (ps[b % 2], 8 * b + 0, bs_t, bst_t,
                     tin[0][:, b, :], tint[:, b, :]),
                    (pq[b % 2], 8 * b + 4, bq_t, bqt_t,
                     v["sqm"][:, b, :], v["sqt"][:, b, :]),
                ):
                    nc.tensor.matmul(P[:, 0:W], wmain, rm[:, 0:W],
                                     start=True, stop=False)
                    nc.tensor.matmul(P[:, 0:W], wtail, rt[:, 0:W],
                                     start=False, stop=True)
                    nc.tensor.matmul(pt[:, tcol:tcol + 4], wmain, rm[:, W:WIDE],
                                     start=True, stop=False)
                    nc.tensor.matmul(pt[:, tcol:tcol + 4], wtail, rt[:, W:WIDE],
                                     start=False, stop=True)
                nc.scalar.copy(v["svb"][:, b, 5:5 + W], ps[b % 2][:, :])
                nc.scalar.copy(v["qvb"][:, b, 5:5 + W], pq[b % 2][:, :])
                nc.scalar.copy(v["svb"][:, b, 5 + W:5 + WIDE],
                               pt[:, 8 * b:8 * b + 4])
                nc.scalar.copy(v["qvb"][:, b, 5 + W:5 + WIDE],
                               pt[:, 8 * b + 4:8 * b + 8])

        def chunk_gen(img, ci, par):
            """Generator emitting this chunk's DVE stream (median, scans,
            dd, reciprocal, formula) + its ACT tail + stores, yielding after
            every DVE op for pairwise interleaving."""
            tin = tins[par]
            out_t = outs[par]
            v = vsets[par]
            r0 = ci * 128 * ROWS

            mid_box = [None]
            yield from emit_median(nc.vector, poolA, tin, 0, W, WIDE, W + 3,
                                   mid_box)

            for b in range(ROWS):
                nc.vector.tensor_tensor_scan(
                    v["s25"][:, b, :], v["svb"][:, b, 5:5 + WIDE],
                    v["svb"][:, b, 0:WIDE], 0.0, ALU.add, ALU.subtract)
                yield
                nc.vector.tensor_tensor_scan(
                    v["q25"][:, b, :], v["qvb"][:, b, 5:5 + WIDE],
                    v["qvb"][:, b, 0:WIDE], epsp_ap, ALU.add, ALU.subtract)
                yield
            # s2 = (sqlam * s25)^2 on ACT; dd = c2*q25 - s2; g = 1/dd
            nc.scalar.activation(v["s2b"][:, :, :], v["s25"][:, :, 4:4 + W],
                                 ACTF.Square, 0.0, sqlam_ap)
            nc.vector.scalar_tensor_tensor(v["ddb"][:, :, :],
                                           v["q25"][:, :, 4:4 + W],
                                           c2_ap, v["s2b"][:, :, :],
                                           ALU.mult, ALU.subtract)
            yield
            nc.vector.reciprocal_approx_fast(out=v["gb"][:, :, :],
                                             in_=v["ddb"][:, :, :])
            yield
            nc.scalar.copy(v["g16"][:, :, :], v["gb"][:, :, :])

            yield from emit_formula(nc, poolA, tin, W, mid_box[0], v["t1b"],
                                    v["g16"], nb_ap, out_t)

            for b in range(ROWS):
                row = img * H + r0 + b * 128
                nc.sync.dma_start(ya[row:row + 128, :], out_t[:, b, :])

        def body():
            for _ in range(repeat):
                for img in range(IMGS_PER_CORE):
                    # chunk pair (the two halves of one image), heads first,
                    # then the DVE streams interleaved op-by-op
                    emit_head(img, 0, 0)
                    emit_head(img, 1, 1)
                    ga = chunk_gen(img, 0, 0)
                    gc = chunk_gen(img, 1, 1)
                    done_a = done_b = False
                    while not (done_a and done_b):
                        if not done_a:
                            try:
                                next(ga)
                            except StopIteration:
                                done_a = True
                        if not done_b:
                            try:
                                next(gc)
                            except StopIteration:
                                done_b = True

        if hw_loop is None:
            body()
        else:
            with tc.For_i(0, hw_loop, 1):
                body()

    global _PEAKS
    _PEAKS = (poolA.peak,)
    nc.compile()
    return nc


_PEAKS = None


_MODULE = None


def _get_module():
    global _MODULE
    if _MODULE is None:
        _MODULE = build_module()
    return _MODULE


def prepare_in_maps(x, noise_var, noise_bias):
    """Host-side prep: fp16 padded shards + banded PE weights + scalars."""
    x = np.ascontiguousarray(np.asarray(x, dtype=np.float32))
    nv = float(np.asarray(noise_var).reshape(-1)[0])
    nb = float(np.asarray(noise_bias).reshape(-1)[0])
    B = x.shape[0]
    assert x.shape == (B, 1, H, W) and B == N_CORES * IMGS_PER_CORE

    xpad = np.zeros((B, H + 4, WIDE), np.float16)
    xpad[:, 2:2 + H, 2:2 + W] = x[:, 0]

    w1h = np.float16(1.0 / np.sqrt(600.0 * nv))
    w1 = float(w1h)
    w2 = float(np.float32(1.0 / (24.0 * nv)))
    k_idx = np.arange(128)[:, None]
    m_idx = np.arange(128)[None, :]
    band = (np.abs(k_idx - 2 - m_idx) <= 2)
    bandt = (np.abs(128 + np.arange(4)[:, None] - 2 - m_idx) <= 2)
    bs = (band * w1).astype(np.float16)
    bst = (bandt * w1).astype(np.float16)
    bq = (band * w2).astype(np.float32)
    bqt = (bandt * w2).astype(np.float32)

    c2 = 1.0 / (24.0 * nv * w2)
    sqlam = np.sqrt(1.0 / (600.0 * nv * w1 * w1))
    eps_p = (1e-10 / nv) / c2
    nvb = np.zeros((128, NVB_COLS), np.float32)
    nvb[:, 0] = nb
    nvb[:, 1] = c2
    nvb[:, 2] = sqlam
    nvb[:, 3] = eps_p

    in_maps = []
    for c in range(N_CORES):
        shard = np.ascontiguousarray(
            xpad[c * IMGS_PER_CORE:(c + 1) * IMGS_PER_CORE])
        in_maps.append({"x": shard, "nvb": nvb, "bs": bs, "bst": bst,
                        "bq": bq, "bqt": bqt})
    return in_maps


def kernel(x, noise_var, noise_bias):
    in_maps = prepare_in_maps(x, noise_var, noise_bias)
    B = N_CORES * IMGS_PER_CORE
    nc = _get_module()
    res = run_bass_kernel_spmd(nc, in_maps, core_ids=list(range(N_CORES)))
    y = np.empty((B, 1, H, W), np.float32)
    for c in range(N_CORES):
        y[c * IMGS_PER_CORE:(c + 1) * IMGS_PER_CORE, 0] = \
            res.results[c]["y"].astype(np.float32)
    return y
